# revision 1
# baseline (speedup 1.0000x reference)
"""BottleneckAdapter (LN -> down-proj -> GELU -> up-proj -> +residual) on 8 trn2 cores.

Data-parallel: x [16,1500,1280] flattened to [24000,1280], 3000 rows/core.
Per-core bass/Tile kernel, token-major tiles of 128 rows:
  - LN folded:  z = x' @ A^T + c   where x' = (x-mean)*rstd (bf16),
    A = gamma * w_down  (gamma folded into the down weights),
    c = w_down @ beta + b_down  (applied as the gelu's per-partition bias).
  - transposes of x' chunks via PE (matmul transpose mode) -> PSUM -> SBUF.
  - down matmul accumulates zT [64,T] over 10 K-chunks of 128 features.
  - gelu (exact erf) on ACT with bias=c, output bf16 in up-matmul lhsT layout.
  - up matmul lhsT = [gelu(z); ones] (65 x T) against [w_up^T; b_up] so the
    output bias rides the contraction; residual add in fp32 on DVE.
"""

import sys

sys.path.insert(0, "/opt/trn_rl_repo")

from contextlib import ExitStack

import ml_dtypes
import numpy as np

import concourse.bacc as bacc
import concourse.bass as bass
import concourse.tile as tile
from concourse import mybir
from concourse.bass_utils import run_bass_kernel_spmd

N_CORES = 8
D_MODEL = 1280
D_BOTTLE = 64
LN_EPS = 1e-5
ROWS_PER_CORE = 16 * 1500 // N_CORES  # 3000
P = 128
N_CHUNKS = D_MODEL // P  # 10
BF16 = mybir.dt.bfloat16
F32 = mybir.dt.float32

# Up-matmul output split (each slice must stay within one 2KB PSUM bank).
UP_SLICES = [(0, 512), (512, 512), (1024, 256)]
# How many up-slices take the residual via an accumulating identity matmul
# on PE (evacuated by ACT copy) instead of a DVE tensor add.
RESID_PE_SLICES = 2


def _build_bass(reps=1, loop_reps=1, mode="full", resid_pe=None, store_ring="sync"):
    if resid_pe is None:
        resid_pe = RESID_PE_SLICES
    do_dma = mode in ("full", "dma", "load", "dma_sync")
    do_store = mode in ("full", "dma", "dma_sync")
    do_compute = mode in ("full", "compute")
    nc = bacc.Bacc(trn_type="TRN2", debug=False)

    x_in = nc.dram_tensor("x", [ROWS_PER_CORE, D_MODEL], F32, kind="ExternalInput")
    at_in = nc.dram_tensor("at", [P, N_CHUNKS * D_BOTTLE], BF16, kind="ExternalInput")
    wut_in = nc.dram_tensor("wut", [D_BOTTLE + 1, D_MODEL], BF16, kind="ExternalInput")
    cvec_in = nc.dram_tensor("cvec", [D_BOTTLE, 1], F32, kind="ExternalInput")
    ident_in = nc.dram_tensor("ident", [P, P], BF16, kind="ExternalInput")
    y_out = nc.dram_tensor("y", [ROWS_PER_CORE, D_MODEL], F32, kind="ExternalOutput")

    with tile.TileContext(nc) as tc, ExitStack() as ctx:
        singles = ctx.enter_context(tc.tile_pool(name="singles", bufs=1))
        xpool = ctx.enter_context(tc.tile_pool(name="xpool", bufs=10))
        xppool = ctx.enter_context(tc.tile_pool(name="xppool", bufs=6))
        xtpool = ctx.enter_context(tc.tile_pool(name="xtpool", bufs=6))
        statpool = ctx.enter_context(tc.tile_pool(name="statpool", bufs=12))
        ypool = ctx.enter_context(tc.tile_pool(name="ypool", bufs=6))
        # PSUM budget is 8 banks: 2 pools x 2 bufs of [128,5,128]bf16
        # transpose staging (1 bank each), 1x [64,128]f32 z-accumulator, and
        # 3x [128,<=512]f32 up-proj slices (1 bank each).
        ps_xt_a = ctx.enter_context(tc.tile_pool(name="ps_xt_a", bufs=2, space="PSUM"))
        ps_xt_b = ctx.enter_context(tc.tile_pool(name="ps_xt_b", bufs=2, space="PSUM"))
        ps_z_pool = ctx.enter_context(tc.tile_pool(name="ps_z", bufs=1, space="PSUM"))
        ps_up_pool = ctx.enter_context(
            tc.tile_pool(name="ps_up", bufs=3, space="PSUM")
        )

        at_sb = singles.tile([P, N_CHUNKS, D_BOTTLE], BF16)
        nc.sync.dma_start(at_sb.rearrange("p c k -> p (c k)"), at_in[:, :])
        wut_sb = singles.tile([D_BOTTLE + 1, D_MODEL], BF16)
        nc.sync.dma_start(wut_sb[:, :], wut_in[:, :])
        cvec_sb = singles.tile([D_BOTTLE, 1], F32)
        nc.sync.dma_start(cvec_sb[:, :], cvec_in[:, :])
        ident_sb = singles.tile([P, P], BF16)
        nc.sync.dma_start(ident_sb[:, :], ident_in[:, :])
        ident_f32 = singles.tile([P, P], F32)
        nc.vector.tensor_copy(out=ident_f32[:, :], in_=ident_sb[:, :])
        # Persistent gelu/ones tiles (rotated manually): row 64 is the ones
        # row for the up-matmul bias trick, written once instead of per tile.
        N_G = 4
        g65s = []
        for gi in range(N_G):
            g = singles.tile([D_BOTTLE + 1, P], BF16, tag=f"g65_{gi}")
            nc.vector.memset(g[D_BOTTLE : D_BOTTLE + 1, :], 1.0)
            g65s.append(g)

        loop_cm = tc.For_i(0, loop_reps, 1) if loop_reps > 1 else None
        if loop_cm is not None:
            loop_cm.__enter__()

        n_tiles = (ROWS_PER_CORE + P - 1) // P
        for it_rep in range(reps * n_tiles):
            it = it_rep % n_tiles
            t0 = it * P
            T = min(P, ROWS_PER_CORE - t0)

            x_t = xpool.tile([P, D_MODEL], F32)
            if do_dma:
                nc.sync.dma_start(x_t[:T, :], x_in[t0 : t0 + T, :])
            else:
                # compute-only variant: give the tile a writer
                nc.vector.memset(x_t[:1, 0:2], 0.0)
            if mode == "noop":
                nc.vector.memset(x_t[:1, 0:2], float(it))
                continue
            if not do_compute:
                # DMA-floor variant: bounce the tile back out (or load-only).
                if do_store:
                    eng = nc.sync if mode == "dma_sync" else nc.gpsimd
                    eng.dma_start(y_out[t0 : t0 + T, :], x_t[:T, :])
                continue

            # LN stats: mean/var per token in one DVE pass (4 bn_stats subtiles).
            stats = statpool.tile([P, 4, 6], F32)
            for j in range(4):
                nc.vector.bn_stats(
                    out=stats[:T, j, :], in_=x_t[:T, j * 320 : (j + 1) * 320]
                )
            mv = statpool.tile([P, 2], F32)
            nc.vector.bn_aggr(out=mv[:T, :], in_=stats[:T, :, :])

            # rstd = rsqrt(var + eps) entirely on DVE (bitcast seed + 2 Newton
            # steps) -- keeping Sqrt off ScalarE lets every ACT func in the
            # kernel (Identity/Copy/Gelu) live in one LUT set, avoiding
            # ~1.3us LoadActFuncSet reloads per tile.
            vv = statpool.tile([P, 1], F32)
            nc.vector.tensor_scalar_add(out=vv[:T, :], in0=mv[:T, 1:2], scalar1=LN_EPS)
            yb = statpool.tile([P, 1], mybir.dt.int32)
            nc.vector.tensor_scalar(
                out=yb[:T, :],
                in0=vv[:T, :].bitcast(mybir.dt.int32),
                scalar1=1,
                scalar2=None,
                op0=mybir.AluOpType.arith_shift_right,
                op1=mybir.AluOpType.bypass,
            )
            nc.vector.tensor_scalar(
                out=yb[:T, :],
                in0=yb[:T, :],
                scalar1=-1,
                scalar2=0x5F3759DF,
                op0=mybir.AluOpType.mult,
                op1=mybir.AluOpType.add,
            )
            rstd = statpool.tile([P, 1], F32)
            y0 = yb[:T, :].bitcast(F32)
            cur = y0
            for newton_i in range(1, 2):
                ysq = statpool.tile([P, 1], F32, tag=f"nt_ysq{newton_i}")
                nc.vector.tensor_tensor(
                    out=ysq[:T, :], in0=cur, in1=cur, op=mybir.AluOpType.mult
                )
                w = statpool.tile([P, 1], F32, tag=f"nt_w{newton_i}")
                nc.vector.scalar_tensor_tensor(
                    out=w[:T, :],
                    in0=ysq[:T, :],
                    scalar=-0.5,
                    in1=vv[:T, :],
                    op0=mybir.AluOpType.mult,
                    op1=mybir.AluOpType.mult,
                )
                nc.vector.tensor_scalar_add(out=w[:T, :], in0=w[:T, :], scalar1=1.5)
                dst = rstd if newton_i == 1 else statpool.tile(
                    [P, 1], F32, tag="nt_y1"
                )
                nc.vector.tensor_tensor(
                    out=dst[:T, :], in0=cur, in1=w[:T, :], op=mybir.AluOpType.mult
                )
                cur = dst[:T, :]
            # x'' = x * rstd, downcast to bf16 (DVE tensor_scalar runs 2x_2P
            # for fp32 SBUF single-src). Mean-centering is folded into the
            # down-projection weights on the host (A~ = A - u*1^T/D), so no
            # bias term is needed here.
            xp = xppool.tile([P, D_MODEL], BF16)
            nc.vector.tensor_scalar_mul(
                out=xp[:T, :], in0=x_t[:T, :], scalar1=rstd[:T, :]
            )

            # Transpose 10 chunks of x' into feature-major layout; two PSUM
            # staging groups of 5 chunks so transposes of group B overlap the
            # evacuation of group A.
            xt_sb = xtpool.tile([P, N_CHUNKS, P], BF16)
            for half, pool_h in ((0, ps_xt_a), (1, ps_xt_b)):
                ps_xt = pool_h.tile([P, N_CHUNKS // 2, P], BF16)
                for cc in range(N_CHUNKS // 2):
                    c = half * (N_CHUNKS // 2) + cc
                    nc.tensor.transpose(
                        ps_xt[:, cc, :T],
                        xp[:T, c * P : (c + 1) * P],
                        ident_sb[:T, :T],
                    )
                nc.scalar.copy(
                    out=xt_sb[:, half * 5 : half * 5 + 5, :], in_=ps_xt[:, :, :]
                )

            # Down-proj: zT[k, t] accumulated over 10 feature chunks.
            ps_z = ps_z_pool.tile([D_BOTTLE, P], F32)
            for c in range(N_CHUNKS):
                nc.tensor.matmul(
                    ps_z[:, :T],
                    at_sb[:, c, :],
                    xt_sb[:, c, :T],
                    start=(c == 0),
                    stop=(c == N_CHUNKS - 1),
                )

            # Exact gelu with folded bias c = w_down@beta + b_down; bf16 out.
            g65 = g65s[it % N_G]
            nc.scalar.activation(
                out=g65[0:D_BOTTLE, :T],
                in_=ps_z[:, :T],
                func=mybir.ActivationFunctionType.Gelu,
                bias=cvec_sb[:, :],
                scale=1.0,
            )

            # Up-proj (+b_up via the ones row) into PSUM fp32, in <=512-col
            # slices (one PSUM bank each). For the first two slices the
            # residual x rides a second accumulating identity-matmul on PE
            # and the slice is evacuated by a plain ACT copy; the last slice
            # adds the residual on DVE. This splits the evacuation cost
            # across ACT/DVE/PE.
            y_t = ypool.tile([P, D_MODEL], F32)
            for si, (n0, nw) in enumerate(UP_SLICES):
                ps_up = ps_up_pool.tile([P, 512], F32)
                resid_on_pe = si < resid_pe
                nc.tensor.matmul(
                    ps_up[:T, :nw],
                    g65[:, :T],
                    wut_sb[:, n0 : n0 + nw],
                    start=True,
                    stop=not resid_on_pe,
                )
                if resid_on_pe:
                    nc.tensor.matmul(
                        ps_up[:T, :nw],
                        ident_f32[:T, :T],
                        x_t[:T, n0 : n0 + nw],
                        start=False,
                        stop=True,
                    )
                    nc.scalar.copy(
                        out=y_t[:T, n0 : n0 + nw], in_=ps_up[:T, :nw]
                    )
                else:
                    nc.vector.tensor_add(
                        out=y_t[:T, n0 : n0 + nw],
                        in0=ps_up[:T, :nw],
                        in1=x_t[:T, n0 : n0 + nw],
                    )
            if do_dma:
                store_eng = {"sync": nc.sync, "scalar": nc.scalar, "gpsimd": nc.gpsimd}[
                    store_ring
                ]
                store_eng.dma_start(y_out[t0 : t0 + T, :], y_t[:T, :])

        if loop_cm is not None:
            loop_cm.__exit__(None, None, None)

    nc.compile()
    return nc


_CACHED_NC = {}


def _get_nc(reps=1, loop_reps=1, mode="full", resid_pe=None, store_ring="sync"):
    key = (reps, loop_reps, mode, resid_pe, store_ring)
    if key not in _CACHED_NC:
        _CACHED_NC[key] = _build_bass(reps, loop_reps, mode, resid_pe, store_ring)
    return _CACHED_NC[key]


def _prep_in_maps(inputs):
    x = np.asarray(inputs["x"], dtype=np.float32).reshape(-1, D_MODEL)
    gamma = np.asarray(inputs["gamma"], dtype=np.float32)
    beta = np.asarray(inputs["beta"], dtype=np.float32)
    w_down = np.asarray(inputs["w_down"], dtype=np.float32)
    b_down = np.asarray(inputs["b_down"], dtype=np.float32)
    w_up = np.asarray(inputs["w_up"], dtype=np.float32)
    b_up = np.asarray(inputs["b_up"], dtype=np.float32)

    # A^T chunks: at[p, c, k] = A~[k, c*128+p] where A = gamma * w_down and
    # A~ = A - rowsum(A)/D folds the LN mean-centering into the weights:
    # sum_f A~[k,f] * x[t,f] * rstd[t] = rstd[t] * sum_f A[k,f] (x[t,f]-mean[t]).
    a_mat = w_down * gamma[None, :]  # [64, 1280]
    a_mat = a_mat - a_mat.sum(axis=1, keepdims=True) / D_MODEL
    at = a_mat.T  # [1280, 64]
    at = at.reshape(N_CHUNKS, P, D_BOTTLE).transpose(1, 0, 2)  # [128, 10, 64]
    at = np.ascontiguousarray(at.reshape(P, N_CHUNKS * D_BOTTLE)).astype(
        ml_dtypes.bfloat16
    )
    wut = np.concatenate([w_up.T, b_up[None, :]], axis=0).astype(
        ml_dtypes.bfloat16
    )  # [65, 1280]
    cvec = (w_down @ beta + b_down).reshape(D_BOTTLE, 1).astype(np.float32)
    ident = np.eye(P, dtype=ml_dtypes.bfloat16)

    in_maps = []
    for i in range(N_CORES):
        shard = np.ascontiguousarray(
            x[i * ROWS_PER_CORE : (i + 1) * ROWS_PER_CORE]
        )
        in_maps.append(
            {"x": shard, "at": at, "wut": wut, "cvec": cvec, "ident": ident}
        )
    return in_maps


def run_with_results(inputs, trace=False, reps=1, loop_reps=1, mode="full", resid_pe=None, store_ring="sync", **kwargs):
    nc = _get_nc(reps, loop_reps, mode, resid_pe, store_ring)
    in_maps = _prep_in_maps(inputs)
    res = run_bass_kernel_spmd(
        nc, in_maps, core_ids=list(range(N_CORES)), trace=trace, **kwargs
    )
    y = np.concatenate([res.results[i]["y"] for i in range(N_CORES)], axis=0)
    y = y.reshape(16, 1500, D_MODEL).astype(np.float32)
    return y, res


def kernel(**inputs):
    y, _ = run_with_results(inputs, trace=False)
    return y

